# revision 1
# baseline (speedup 1.0000x reference)
"""Trainium2 Bass kernel for nn_Attention_17489106830121.

Math: the reference's einsums sum out entire axes, making attention logits
rank-1: attn[b,h,n,j] = s[b,n,h] * ks[b,j], with
  s  = x @ wqs              (wqs folds Wq head-colsums * SCALE * lksum)
  ks = LN(conv(x)) @ wk     (wk = even-col sums of Wkv)
  vs = LN(conv(x)) @ wv     (odd-col sums)
  out[b,n,:] = (softmax-weighted vs means over j, per (n,h)) @ Mmat + bproj

Per core: one batch element (8 cores == B). Engines:
  PE:    x transpose, conv-as-matmul, score build (h<NH_PE), final matmul
  GPSIMD: score build (h>=NH_PE) via tensor_scalar
  ScalarE: wide Exp activations
  DVE:   LayerNorm, numerator/denominator segmented reductions
"""

import numpy as np

B, N, C, HEADS, SR = 8, 4096, 256, 8, 4
HC = C // HEADS          # 32
SCALE = HC ** -0.5
EPS = 1e-5
HS = 64 // SR            # 16
N2 = HS * HS             # 256
NB = N // 128            # 32 row tiles

_NC_CACHE = {}


def _build_nc(x_eng="gpsimd", scatter_y=True):
    import concourse.bass as bass
    import concourse.bacc as bacc
    import concourse.mybir as mybir
    from concourse import tile

    dt = mybir.dt
    f32, bf16 = dt.float32, dt.bfloat16
    AF = mybir.ActivationFunctionType
    ALU = mybir.AluOpType
    AX = mybir.AxisListType

    nc = bacc.Bacc(None, target_bir_lowering=False)

    x_d = nc.dram_tensor("x", [N, C], f32, kind="ExternalInput")
    ws_d = nc.dram_tensor("wsr", [SR * SR * C, C], f32, kind="ExternalInput")
    wqs_d = nc.dram_tensor("wqs", [C, HEADS], f32, kind="ExternalInput")
    wkv_d = nc.dram_tensor("wkv2", [C, 2], f32, kind="ExternalInput")
    mm_d = nc.dram_tensor("mmat", [HEADS, C], f32, kind="ExternalInput")
    br_d = nc.dram_tensor("brow", [4, C], f32, kind="ExternalInput")
    id_d = nc.dram_tensor("ident", [128, 128], f32, kind="ExternalInput")
    y_d = nc.dram_tensor("y", [N, C], f32, kind="ExternalOutput")

    with tile.TileContext(nc) as tc:
        with (
            tc.tile_pool(name="const", bufs=1) as cp,
            tc.tile_pool(name="xg", bufs=2) as xg,
            tc.tile_pool(name="ep", bufs=2) as ep,
            tc.tile_pool(name="scp", bufs=2) as scp,
            tc.tile_pool(name="wp", bufs=3) as wp,
        ):
            xs = cp.tile([128, NB, C], f32)        # x natural, n%128 on partitions
            wssb = cp.tile([128, 32, C], f32)      # Wsr_flat k-tiles
            xT = cp.tile([128, 2, N], f32)         # x transposed, c on partitions
            wqssb = cp.tile([128, 2, HEADS], f32)
            wkvsb = cp.tile([128, 2, 2], f32)
            mmsb = cp.tile([HEADS, C], f32)
            bsr_r = cp.tile([1, C], f32)
            gam_r = cp.tile([1, C], f32)
            bet_r = cp.tile([1, C], f32)
            bpr_r = cp.tile([1, C], f32)
            vs_r = cp.tile([1, N2], f32)
            idsb = cp.tile([128, 128], f32)
            ones_row = cp.tile([1, 128], f32)
            eps_sb = cp.tile([128, 1], f32)
            s_sb = cp.tile([128, NB, HEADS], f32)
            t_sb = cp.tile([128, NB * HEADS], f32)
            tsc1 = cp.tile([128, NB * HEADS], f32)
            tsc2 = cp.tile([128, NB * HEADS], f32)
            xcv = cp.tile([128, 2, N2], f32)
            xm = cp.tile([128, 2, N2], f32)
            xn = cp.tile([128, 2, N2], f32)
            xnT = cp.tile([128, 2, N2], f32)
            ks_r = cp.tile([1, N2], f32)
            bsrep = cp.tile([128, N2], f32)
            garep = cp.tile([128, N2], f32)
            berep = cp.tile([128, N2], f32)
            ksrep = cp.tile([128, N2], f32)
            vrep8 = cp.tile([128, HEADS, N2], bf16)
            negrow = cp.tile([1, N2], f32)
            kmx = cp.tile([1, 1], f32)
            kmxneg = cp.tile([1, 1], f32)
            kmneg = cp.tile([1, 1], f32)
            nkmx = cp.tile([128, 1], f32)
            nkmn = cp.tile([128, 1], f32)
            mu = cp.tile([128, 2], f32)
            var = cp.tile([128, 2], f32)
            logv = cp.tile([128, 2], f32)
            rstd = cp.tile([128, 2], f32)
            sq = cp.tile([128, N2], f32)
            den = cp.tile([128, NB, HEADS], f32)
            num = cp.tile([128, NB, HEADS], f32)
            dinv = cp.tile([128, NB, HEADS], f32)
            wvc = cp.tile([128, NB, HEADS], f32)

            nc.sync.dma_start(xs[:], x_d[:].rearrange("(t p) c -> p t c", p=128))
            nc.sync.dma_start(wssb[:], ws_d[:].rearrange("(t p) c -> p t c", p=128))
            nc.sync.dma_start(wqssb[:], wqs_d[:].rearrange("(t p) h -> p t h", p=128))
            nc.sync.dma_start(wkvsb[:], wkv_d[:].rearrange("(t p) h -> p t h", p=128))
            nc.sync.dma_start(mmsb[:], mm_d[:])
            nc.sync.dma_start(bsr_r[:], br_d[0:1, :])
            nc.sync.dma_start(gam_r[:], br_d[1:2, :])
            nc.sync.dma_start(bet_r[:], br_d[2:3, :])
            nc.sync.dma_start(bpr_r[:], br_d[3:4, :])
            nc.sync.dma_start(idsb[:], id_d[:])
            nc.vector.memset(ones_row[:], 1.0)
            nc.vector.memset(eps_sb[:], EPS)

            # ---- Phase A: transpose x (c onto partitions), s = x @ wqs ----
            with tc.tile_pool(name="psA", bufs=2, space="PSUM") as pA:
                for g in range(32):
                    tp = pA.tile([128, 2, 128], f32)
                    ct, t0 = divmod(g * 2, NB)
                    for q in range(2):
                        nc.tensor.transpose(
                            tp[:, q, :], xs[:, t0 + q, 128 * ct:128 * (ct + 1)],
                            idsb[:],
                        )
                    # scatter pair (t0, t0+1) into patch-major (pos) order:
                    # source flat (q0, a, pw, kw) == kh-major since kh=2*q0+a
                    v2 = xT[:, ct, :].rearrange(
                        "p (kh kw ph pw) -> p kh kw ph pw", kh=4, kw=4, ph=16, pw=16
                    )
                    dst = v2[:, :, :, t0 // 2, :].rearrange("p kh kw pw -> p kh pw kw")
                    srcv = tp[:].rearrange(
                        "p q i -> p (q i)"
                    ).rearrange("p (kha pw kw) -> p kha pw kw", kha=4, pw=16, kw=4)
                    eng = nc.vector.tensor_copy if g % 2 == 0 else nc.scalar.copy
                    eng(dst, srcv)
                sps = pA.tile([128, NB, HEADS], f32)
                for nb in range(NB):
                    for ct in range(2):
                        nc.tensor.matmul(
                            sps[:, nb, :],
                            lhsT=xT[:, ct, 128 * nb:128 * (nb + 1)],
                            rhs=wqssb[:, ct, :],
                            start=(ct == 0),
                            stop=(ct == 1),
                        )
                nc.vector.tensor_copy(s_sb[:], sps[:])

            # ---- Phase B: conv (kernel=stride=4) + LayerNorm ----
            with tc.tile_pool(name="psB", bufs=2, space="PSUM") as pB:
                brp = pB.tile([128, N2], f32)
                nc.tensor.matmul(brp[:], lhsT=ones_row[:], rhs=bsr_r[:],
                                 start=True, stop=True)
                nc.vector.tensor_copy(bsrep[:], brp[:])
                grp = pB.tile([128, N2], f32)
                nc.tensor.matmul(grp[:], lhsT=ones_row[:], rhs=gam_r[:],
                                 start=True, stop=True)
                nc.vector.tensor_copy(garep[:], grp[:])
                bep = pB.tile([128, N2], f32)
                nc.tensor.matmul(bep[:], lhsT=ones_row[:], rhs=bet_r[:],
                                 start=True, stop=True)
                nc.vector.tensor_copy(berep[:], bep[:])
                for m in range(2):
                    cps = pB.tile([128, N2], f32)
                    for kh in range(4):
                        for kw in range(4):
                            for ct in range(2):
                                kidx = kh * 8 + kw * 2 + ct
                                base = (kh * 4 + kw) * 256 + 128 * m
                                nc.tensor.matmul(
                                    cps[:],
                                    lhsT=xT[:, ct, base:base + 128],
                                    rhs=wssb[:, kidx, :],
                                    start=(kidx == 0),
                                    stop=(kidx == 31),
                                )
                    nc.vector.tensor_tensor(xcv[:, m, :], cps[:], bsrep[:], ALU.add)

                for m in range(2):
                    nc.vector.reduce_sum(mu[:, m:m + 1], xcv[:, m, :], axis=AX.X)
                    nc.vector.tensor_scalar(
                        mu[:, m:m + 1], mu[:, m:m + 1], 1.0 / N2, None, ALU.mult
                    )
                    nc.vector.tensor_scalar(
                        xm[:, m, :], xcv[:, m, :], mu[:, m:m + 1], None, ALU.subtract
                    )
                    nc.vector.tensor_tensor(sq[:], xm[:, m, :], xm[:, m, :],
                                            ALU.mult)
                    nc.vector.reduce_sum(var[:, m:m + 1], sq[:], axis=AX.X)
                    # rstd = exp(-0.5*ln(var/N2 + eps)); Ln+Exp share a table set
                    nc.scalar.activation(
                        logv[:, m:m + 1], var[:, m:m + 1], AF.Ln,
                        bias=eps_sb[:], scale=1.0 / N2,
                    )
                    nc.scalar.activation(
                        rstd[:, m:m + 1], logv[:, m:m + 1], AF.Exp, scale=-0.5
                    )
                    nc.vector.tensor_scalar(
                        xn[:, m, :], xm[:, m, :], rstd[:, m:m + 1], None, ALU.mult
                    )
                    nc.vector.tensor_tensor(xm[:, m, :], xn[:, m, :], garep[:], ALU.mult)
                    nc.vector.tensor_tensor(xn[:, m, :], xm[:, m, :], berep[:], ALU.add)

            # ---- Phase C: ks/vs, score-shift t, transposed (s,t) pairs ----
            with tc.tile_pool(name="psC", bufs=1, space="PSUM") as pC:
                tp2 = pC.tile([128, 4, 128], f32)
                for t2 in range(2):
                    for ct in range(2):
                        nc.tensor.transpose(
                            tp2[:, t2 * 2 + ct, :],
                            xn[:, t2, 128 * ct:128 * (ct + 1)],
                            idsb[:],
                        )
                for t2 in range(2):
                    for ct in range(2):
                        nc.vector.tensor_copy(
                            xnT[:, ct, 128 * t2:128 * (t2 + 1)], tp2[:, t2 * 2 + ct, :]
                        )
                kps_k = pC.tile([1, N2], f32)
                kps_v = pC.tile([1, N2], f32)
                for ct in range(2):
                    nc.tensor.matmul(
                        kps_k[:], lhsT=wkvsb[:, ct, 0:1], rhs=xnT[:, ct, :],
                        start=(ct == 0), stop=(ct == 1),
                    )
                    nc.tensor.matmul(
                        kps_v[:], lhsT=wkvsb[:, ct, 1:2], rhs=xnT[:, ct, :],
                        start=(ct == 0), stop=(ct == 1),
                    )
                nc.vector.tensor_copy(ks_r[:], kps_k[:])
                nc.vector.tensor_copy(vs_r[:], kps_v[:])
                nc.vector.reduce_max(kmx[:], ks_r[:], axis=AX.X)
                nc.vector.tensor_scalar(negrow[:], ks_r[:], -1.0, None, ALU.mult)
                nc.vector.reduce_max(kmxneg[:], negrow[:], axis=AX.X)
                nc.vector.tensor_scalar(kmneg[:], kmx[:], -1.0, None, ALU.mult)

                r1 = pC.tile([128, 1], f32)
                nc.tensor.matmul(r1[:], lhsT=ones_row[:], rhs=kmneg[:],
                                 start=True, stop=True)
                nc.vector.tensor_copy(nkmx[:], r1[:])
                r2 = pC.tile([128, 1], f32)
                nc.tensor.matmul(r2[:], lhsT=ones_row[:], rhs=kmxneg[:],
                                 start=True, stop=True)
                nc.vector.tensor_copy(nkmn[:], r2[:])
                r3 = pC.tile([128, N2], f32)
                nc.tensor.matmul(r3[:], lhsT=ones_row[:], rhs=ks_r[:],
                                 start=True, stop=True)
                nc.vector.tensor_copy(ksrep[:], r3[:])
                r4 = pC.tile([128, N2], f32)
                nc.tensor.matmul(r4[:], lhsT=ones_row[:], rhs=vs_r[:],
                                 start=True, stop=True)
                nc.vector.tensor_copy(vrep8[:, 0, :], r4[:])
                for i in range(1, HEADS):
                    nc.vector.tensor_copy(vrep8[:, i, :], vrep8[:, 0, :])

                s_flat = s_sb[:].rearrange("p nb h -> p (nb h)")
                # t = -max(s*kmax, s*kmin) = min(s*(-kmax), s*(-kmin))
                nc.vector.tensor_scalar(tsc1[:], s_flat, nkmx[:], None, ALU.mult)
                nc.vector.tensor_scalar(tsc2[:], s_flat, nkmn[:], None, ALU.mult)
                nc.vector.tensor_tensor(t_sb[:], tsc1[:], tsc2[:], ALU.min)

            # ---- Main loop: scores -> exp -> num/den -> wv -> y ----
            with tc.tile_pool(name="psD", bufs=2, space="PSUM") as pD:
                for nb in range(NB):
                    Xg = xg.tile([128, HEADS, N2], f32)
                    ts_eng = (nc.gpsimd.tensor_scalar if x_eng == "gpsimd"
                              else nc.vector.tensor_scalar)
                    for h in range(HEADS):
                        j = nb * HEADS + h
                        ts_eng(
                            Xg[:, h, :],
                            ksrep[:],
                            s_sb[:, nb, h:h + 1],
                            t_sb[:, j:j + 1],
                            ALU.mult,
                            ALU.add,
                        )
                    E = ep.tile([128, HEADS, N2], bf16)
                    nc.scalar.activation(E[:], Xg[:], AF.Exp)
                    scr = scp.tile([128, HEADS, N2], bf16)
                    nc.vector.tensor_tensor(scr[:], E[:], vrep8[:], ALU.mult)
                    nc.vector.reduce_sum(num[:, nb, :], scr[:], axis=AX.X)
                    nc.vector.reduce_sum(den[:, nb, :], E[:], axis=AX.X)
                    nc.vector.reciprocal(dinv[:, nb, :], den[:, nb, :])
                    nc.vector.tensor_tensor(
                        wvc[:, nb, :], num[:, nb, :], dinv[:, nb, :], ALU.mult
                    )
                    wt = pD.tile([HEADS, 128], f32)
                    nc.tensor.transpose(wt[:], wvc[:, nb, :], idsb[:])
                    wts = wp.tile([HEADS, 128], f32)
                    nc.vector.tensor_copy(wts[:], wt[:])
                    yp = pD.tile([128, C], f32)
                    nc.tensor.matmul(yp[:], lhsT=wts[:], rhs=mmsb[:],
                                     start=True, stop=False)
                    nc.tensor.matmul(yp[:], lhsT=ones_row[:], rhs=bpr_r[:],
                                     start=False, stop=True)
                    ysb = wp.tile([128, C], f32)
                    eng = nc.vector.tensor_copy if nb % 2 == 0 else nc.scalar.copy
                    eng(ysb[:], yp[:])
                    if scatter_y:
                        # pos-block nb: kh,kw = divmod(nb//2,4); ph in 8*(nb%2)+[0,8)
                        kh, kw = divmod(nb // 2, 4)
                        y5 = y_d[:].rearrange(
                            "(ph q pw r) c -> ph q pw r c", ph=16, q=4, pw=16, r=4
                        )
                        nc.sync.dma_start(
                            y5[8 * (nb % 2):8 * (nb % 2) + 8, kh, :, kw, :], ysb[:]
                        )
                    else:
                        nc.sync.dma_start(y_d[128 * nb:128 * (nb + 1), :], ysb[:])

    nc.compile()
    return nc


def _host_precompute(Wq, Wkv, Wsr, bsr, gamma, beta, Wproj, bproj, k_learn, v_learn):
    lksum = k_learn.reshape(HEADS, HC).sum(1)
    wqs = (Wq.reshape(C, HEADS, HC).sum(2) * (SCALE * lksum)[None, :]).astype(np.float32)
    wkv2 = np.stack([Wkv[:, 0::2].sum(1), Wkv[:, 1::2].sum(1)], 1).astype(np.float32)
    lv = v_learn.reshape(HEADS, HC)
    # out rearrange 'b d n c -> b n (c d)': column index = ci*HEADS + h
    Mmat = np.zeros((HEADS, C), np.float32)
    for h in range(HEADS):
        Mmat[h] = lv[h] @ Wproj[h::HEADS]
    Wsr_flat = np.ascontiguousarray(
        Wsr.transpose(2, 3, 1, 0).reshape(SR * SR * C, C)
    ).astype(np.float32)
    brow = np.stack([bsr, gamma, beta, bproj]).astype(np.float32)
    ident = np.eye(128, dtype=np.float32)
    return dict(wsr=Wsr_flat, wqs=wqs, wkv2=wkv2, mmat=Mmat, brow=brow, ident=ident)


def _pos_perm():
    pos = np.arange(N)
    khkw, rem = pos // 256, pos % 256
    kh, kw = khkw // 4, khkw % 4
    ph, pw = rem // 16, rem % 16
    return 256 * ph + 64 * kh + 4 * pw + kw  # PERM[pos] = natural n


X_ENG = "gpsimd"
SCATTER_Y = True


def kernel(**inputs):
    x = np.asarray(inputs["x"], np.float32)
    weights = _host_precompute(
        *[np.asarray(inputs[k], np.float32) for k in
          ("Wq", "Wkv", "Wsr", "bsr", "gamma", "beta", "Wproj", "bproj",
           "k_learn", "v_learn")]
    )
    key = (X_ENG, SCATTER_Y)
    if key not in _NC_CACHE:
        _NC_CACHE[key] = _build_nc(*key)
    nc = _NC_CACHE[key]
    in_maps = [
        {"x": np.ascontiguousarray(x[i]), **weights} for i in range(B)
    ]
    from concourse.bass_utils import run_bass_kernel_spmd

    res = run_bass_kernel_spmd(nc, in_maps, core_ids=list(range(B)))
    y = np.stack([res.results[i]["y"] for i in range(B)], 0)
    if not SCATTER_Y:
        yn = np.empty_like(y)
        yn[:, _pos_perm(), :] = y
        y = yn
    return y

